# revision 14
# baseline (speedup 1.0000x reference)
"""Trainium2 Bass kernel for CapsNet dynamic routing (nn_Capsule_13692355740297).

Math (per batch element):
    u_hat[i, (n,d)] = u[i, :] @ W[:, (n,d)]            # never materialized
    iter1: c uniform 1/10  -> s1 = 0.1 * (sum_i u_i)^T W
    iter k: b[i, n] = v_n . u_i   with v_n = W_n o_n   # contract Din on PE
            c = softmax_n(b)                           # free-dim softmax, [i,n] layout
            R[n, :] = sum_i c[i, n] u_i                # contract i on PE
            s[n, :] = R[n, :] @ W_n                    # small fixup matmuls
            o = squash(s)
Sharding: data-parallel over batch, 8 batch elements per core, no collectives.

SBUF per core: U f32 [128, 8*4096] (natural, i%128 on partitions) = 128KB/part
               UT bf16 [128, 8*4096] (Din on partitions)           = 64KB/part
UT is produced on-chip with PE identity-transposes (f32 DMA transpose is not
supported by the xbar), cast to bf16 in the PSUM->SBUF copy.
"""

import numpy as np

B, I_FULL, DIN = 64, 4096, 128
NCAP, DCAP = 10, 16
KND = NCAP * DCAP  # 160
NCORES = 8
BC = B // NCORES  # 8 batch elements per core
NT_FULL = I_FULL // 128  # 32 i-tiles per batch
EPS = 1e-7


def build_nc(bc=BC, nt=NT_FULL):
    import concourse.bacc as bacc
    import concourse.mybir as mybir
    from concourse.tile import TileContext

    fp32 = mybir.dt.float32
    bf16 = mybir.dt.bfloat16
    AX = mybir.AxisListType
    ALU = mybir.AluOpType
    ACTF = mybir.ActivationFunctionType

    il = nt * 128  # I per batch

    nc = bacc.Bacc(trn_type="TRN2")
    u_h = nc.dram_tensor("u", [bc, il, DIN], fp32, kind="ExternalInput")
    w_h = nc.dram_tensor("w", [DIN, KND], fp32, kind="ExternalInput")
    ident_h = nc.dram_tensor("ident", [128, 128], fp32, kind="ExternalInput")
    ones_h = nc.dram_tensor("ones", [128, 1], fp32, kind="ExternalInput")
    wt_hi_h = nc.dram_tensor("wt_hi", [128, DIN], bf16, kind="ExternalInput")
    wt_lo_h = nc.dram_tensor("wt_lo", [32, DIN], bf16, kind="ExternalInput")
    m_hi_h = nc.dram_tensor("m_hi", [128, NCAP], bf16, kind="ExternalInput")
    m_lo_h = nc.dram_tensor("m_lo", [32, NCAP], bf16, kind="ExternalInput")
    esel_h = nc.dram_tensor("esel", [128, bc * bc], fp32, kind="ExternalInput")
    out_h = nc.dram_tensor("out", [bc, KND], fp32, kind="ExternalOutput")

    with TileContext(nc) as tc:
        with (
            tc.tile_pool(name="big", bufs=1) as big,
            tc.tile_pool(name="sb2", bufs=2) as sb2,
            tc.tile_pool(name="psT", bufs=2, space="PSUM") as psT,
            tc.tile_pool(name="psB", bufs=2, space="PSUM") as psB,
            tc.tile_pool(name="psR", bufs=2, space="PSUM") as psR,
            tc.tile_pool(name="psS", bufs=2, space="PSUM") as psS,
        ):
            # ---------- persistent SBUF ----------
            U = big.tile([128, bc * il], fp32, name="U_sb")       # [i%128, (b,j,d)]
            UT = big.tile([128, bc * il], bf16, name="UT_sb")     # [d, (b,i)]
            W_sb = big.tile([128, KND], fp32, name="W_sb")
            ident = big.tile([128, 128], fp32, name="ident_sb")
            ones = big.tile([128, 1], fp32, name="ones_sb")
            wt_hi = big.tile([128, DIN], bf16, name="wt_hi_sb")
            wt_lo = big.tile([32, DIN], bf16, name="wt_lo_sb")
            m_hi = big.tile([128, NCAP], bf16, name="m_hi_sb")
            m_lo = big.tile([32, NCAP], bf16, name="m_lo_sb")
            esel = big.tile([128, bc * bc], fp32, name="esel_sb")
            r0 = big.tile([128, bc], fp32, name="r0_sb")
            s_all = big.tile([bc, KND], fp32, name="s_all_sb")
            o_all = big.tile([bc, KND], fp32, name="o_all_sb")

            # ---------- constant loads ----------
            nc.sync.dma_start(out=W_sb[:, :], in_=w_h.ap())
            nc.sync.dma_start(out=ident[:, :], in_=ident_h.ap())
            nc.sync.dma_start(out=ones[:, :], in_=ones_h.ap())
            nc.sync.dma_start(out=wt_hi[:, :], in_=wt_hi_h.ap())
            nc.sync.dma_start(out=wt_lo[:, :], in_=wt_lo_h.ap())
            nc.sync.dma_start(out=m_hi[:, :], in_=m_hi_h.ap())
            nc.sync.dma_start(out=m_lo[:, :], in_=m_lo_h.ap())
            nc.sync.dma_start(out=esel[:, :], in_=esel_h.ap())

            Uv = U[:, :].rearrange("p (b j d) -> p b j d", b=bc, j=nt, d=128)
            UTv = UT[:, :].rearrange("p (b i) -> p b i", b=bc, i=il)

            # ---------- bulk load: u[b] as [128, nt, 128] (i%128 on partitions) ----------
            for b in range(bc):
                nc.sync.dma_start(
                    out=Uv[:, b],
                    in_=u_h.ap()[b].rearrange("(j p) d -> p j d", p=128),
                )

            # ---------- phase 1: build UT (PE transpose + cast), r0, s1 ----------
            for b in range(bc):
                for j in range(nt):
                    tp = psT.tile([128, 128], fp32, name="tp", tag="tp")
                    nc.tensor.transpose(tp[:, :], Uv[:, b, j], ident[:, :])
                    dst = UTv[:, b, 128 * j : 128 * (j + 1)]
                    if j % 2 == 0:
                        nc.scalar.copy(out=dst, in_=tp[:, :])
                    else:
                        nc.vector.tensor_copy(out=dst, in_=tp[:, :])
                # r0[:, b] = sum_i u_i (from UT, free-dim reduce)
                nc.vector.reduce_sum(
                    out=r0[:, b : b + 1], in_=UTv[:, b], axis=AX.X, op=ALU.add
                )
            # s1 (all batches at once) = 0.1 * r0^T W -> [bc, 160]
            s1p = psS.tile([bc, KND], fp32, name="s1p", tag="sacc")
            nc.tensor.matmul(s1p[:, :], r0[:, :], W_sb[:, :])
            nc.scalar.mul(out=s_all[:, :], in_=s1p[:, :], mul=0.1)

            def squash(it):
                """s_all -> o_all, in place over [bc, 160]."""
                sq = sb2.tile([bc, KND], fp32, name=f"sq{it}", tag="sq")
                q = sb2.tile([bc, NCAP], fp32, name=f"q{it}", tag="q")
                rt = sb2.tile([bc, NCAP], fp32, name=f"rt{it}", tag="rt")
                den = sb2.tile([bc, NCAP], fp32, name=f"den{it}", tag="den")
                rden = sb2.tile([bc, NCAP], fp32, name=f"rden{it}", tag="rden")
                coef = sb2.tile([bc, NCAP], fp32, name=f"coef{it}", tag="coef")
                nc.vector.tensor_tensor(
                    out=sq[:, :], in0=s_all[:, :], in1=s_all[:, :], op=ALU.mult
                )
                nc.vector.reduce_sum(
                    out=q[:, :],
                    in_=sq[:, :].rearrange("b (n d) -> b n d", n=NCAP),
                    axis=AX.X,
                    op=ALU.add,
                )
                nc.vector.tensor_scalar_add(q[:, :], q[:, :], EPS)
                nc.scalar.sqrt(out=rt[:, :], in_=q[:, :])
                nc.vector.tensor_scalar_add(den[:, :], q[:, :], 1.0)
                nc.vector.reciprocal(out=rden[:, :], in_=den[:, :])
                nc.vector.tensor_tensor(
                    out=coef[:, :], in0=rt[:, :], in1=rden[:, :], op=ALU.mult
                )
                nc.vector.tensor_tensor(
                    out=o_all[:, :].rearrange("b (n d) -> b n d", n=NCAP),
                    in0=s_all[:, :].rearrange("b (n d) -> b n d", n=NCAP),
                    in1=coef[:, :].unsqueeze(2).broadcast_to([bc, NCAP, DCAP]),
                    op=ALU.mult,
                )

            def make_V(it):
                """o_all -> V_sb bf16 [128, bc*10] (V^T per batch)."""
                oth_p = psT.tile([128, bc], fp32, name=f"oth{it}", tag="tp")
                otl_p = psT.tile([32, bc], fp32, name=f"otl{it}", tag="tp")
                nc.tensor.transpose(oth_p[:, :], o_all[:, 0:128], ident[:bc, :bc])
                nc.tensor.transpose(otl_p[:, :], o_all[:, 128:KND], ident[:bc, :bc])
                oth = sb2.tile([128, bc], bf16, name=f"oth_s{it}", tag="oth_s")
                otl = sb2.tile([32, bc], bf16, name=f"otl_s{it}", tag="otl_s")
                nc.scalar.copy(out=oth[:, :], in_=oth_p[:, :])
                nc.scalar.copy(out=otl[:, :], in_=otl_p[:, :])
                # Oexp[k, (b,n)] = oT[k, b] * M[k, n]
                oeh = sb2.tile([128, bc * NCAP], bf16, name=f"oeh{it}", tag="oeh")
                oel = sb2.tile([32, bc * NCAP], bf16, name=f"oel{it}", tag="oel")
                nc.vector.tensor_tensor(
                    out=oeh[:, :].rearrange("p (b n) -> p b n", b=bc),
                    in0=oth[:, :].unsqueeze(2).broadcast_to([128, bc, NCAP]),
                    in1=m_hi[:, :].unsqueeze(1).broadcast_to([128, bc, NCAP]),
                    op=ALU.mult,
                )
                nc.vector.tensor_tensor(
                    out=oel[:, :].rearrange("p (b n) -> p b n", b=bc),
                    in0=otl[:, :].unsqueeze(2).broadcast_to([32, bc, NCAP]),
                    in1=m_lo[:, :].unsqueeze(1).broadcast_to([32, bc, NCAP]),
                    op=ALU.mult,
                )
                vp = psT.tile([128, bc * NCAP], fp32, name=f"vp{it}", tag="tp")
                nc.tensor.matmul(vp[:, :], wt_hi[:, :], oeh[:, :], start=True, stop=False)
                nc.tensor.matmul(vp[:, :], wt_lo[:, :], oel[:, :], start=False, stop=True)
                V = sb2.tile([128, bc * NCAP], bf16, name=f"V{it}", tag="V")
                nc.scalar.copy(out=V[:, :], in_=vp[:, :])
                return V

            squash(1)

            # ---------- routing iterations 2..3 ----------
            for it in (2, 3):
                V = make_V(it)
                sp_all = psS.tile([bc, KND], fp32, name=f"sp{it}", tag="sacc")
                for b in range(bc):
                    # agreement logits b^T[i, n], tiled over i
                    btp = psB.tile([128, nt * NCAP], fp32, name=f"btp{it}", tag="btp")
                    for j in range(nt):
                        nc.tensor.matmul(
                            btp[:, NCAP * j : NCAP * (j + 1)],
                            UTv[:, b, 128 * j : 128 * (j + 1)],
                            V[:, NCAP * b : NCAP * (b + 1)],
                        )
                    # softmax over n (free dim), no max-subtraction (|b| bounded)
                    eb = sb2.tile([128, nt * NCAP], fp32, name=f"eb{it}", tag="eb")
                    nc.scalar.activation(eb[:, :], btp[:, :], ACTF.Exp)
                    ebv = eb[:, :].rearrange("p (j n) -> p j n", j=nt)
                    Z = sb2.tile([128, nt], fp32, name=f"Z{it}", tag="Z")
                    nc.vector.reduce_sum(out=Z[:, :], in_=ebv, axis=AX.X, op=ALU.add)
                    rZ = sb2.tile([128, nt], fp32, name=f"rZ{it}", tag="rZ")
                    nc.vector.reciprocal(out=rZ[:, :], in_=Z[:, :])
                    cc = sb2.tile([128, nt * NCAP], fp32, name=f"cc{it}", tag="cc")
                    nc.vector.tensor_tensor(
                        out=cc[:, :].rearrange("p (j n) -> p j n", j=nt),
                        in0=ebv,
                        in1=rZ[:, :].unsqueeze(2).broadcast_to([128, nt, NCAP]),
                        op=ALU.mult,
                    )
                    # R[n, :] = sum_i c[i, n] u_i  (PSUM-accumulated over i tiles)
                    Rp = psR.tile([NCAP, DIN], fp32, name=f"Rp{it}", tag="Rp")
                    for j in range(nt):
                        nc.tensor.matmul(
                            Rp[:, :],
                            cc[:, NCAP * j : NCAP * (j + 1)],
                            Uv[:, b, j],
                            start=(j == 0),
                            stop=(j == nt - 1),
                        )
                    R_sb = sb2.tile([NCAP, DIN], fp32, name=f"R{it}", tag="R")
                    nc.scalar.copy(out=R_sb[:, :], in_=Rp[:, :])
                    RTp = psT.tile([128, NCAP], fp32, name=f"RTp{it}", tag="tp")
                    nc.tensor.transpose(RTp[:, :], R_sb[:, :], ident[:NCAP, :NCAP])
                    RT = sb2.tile([128, NCAP], fp32, name=f"RT{it}", tag="RT")
                    nc.scalar.copy(out=RT[:, :], in_=RTp[:, :])
                    # s[(n,d)] = sum_d' RT[d', n] W[d', (n,d)]
                    prod = sb2.tile([128, KND], fp32, name=f"prod{it}", tag="prod")
                    nc.vector.tensor_tensor(
                        out=prod[:, :].rearrange("p (n d) -> p n d", n=NCAP),
                        in0=RT[:, :].unsqueeze(2).broadcast_to([128, NCAP, DCAP]),
                        in1=W_sb[:, :].rearrange("p (n d) -> p n d", n=NCAP),
                        op=ALU.mult,
                    )
                    nc.tensor.matmul(
                        sp_all[:, :],
                        esel[:, bc * b : bc * (b + 1)],
                        prod[:, :],
                        start=(b == 0),
                        stop=(b == bc - 1),
                    )
                nc.scalar.copy(out=s_all[:, :], in_=sp_all[:, :])
                squash(it)

            # ---------- store ----------
            nc.sync.dma_start(out=out_h.ap(), in_=o_all[:, :])

    nc.compile()
    return nc


def make_const_inputs(bc=BC):
    import ml_dtypes

    ident = np.eye(128, dtype=np.float32)
    ones = np.ones((128, 1), dtype=np.float32)
    mask = np.zeros((KND, NCAP), dtype=np.float32)
    for k in range(KND):
        mask[k, k // DCAP] = 1.0
    esel = np.zeros((128, bc * bc), dtype=np.float32)
    for b in range(bc):
        esel[:, b * bc + b] = 1.0
    return {
        "ident": ident,
        "ones": ones,
        "m_hi": mask[:128].astype(ml_dtypes.bfloat16),
        "m_lo": mask[128:].astype(ml_dtypes.bfloat16),
        "esel": esel,
    }


def make_w_inputs(W):
    import ml_dtypes

    W = np.asarray(W, dtype=np.float32)
    WT = W.T.copy()  # [160, 128]
    return {
        "w": W,
        "wt_hi": WT[:128].astype(ml_dtypes.bfloat16),
        "wt_lo": WT[128:].astype(ml_dtypes.bfloat16),
    }


_CACHE = {}


def kernel(u_vecs, W):
    from concourse import bass_utils

    u_vecs = np.asarray(u_vecs, dtype=np.float32)
    W = np.asarray(W, dtype=np.float32)
    if "nc" not in _CACHE:
        _CACHE["nc"] = build_nc()
    nc = _CACHE["nc"]

    consts = make_const_inputs()
    wis = make_w_inputs(W)
    in_maps = []
    for c in range(NCORES):
        m = {"u": np.ascontiguousarray(u_vecs[c * BC : (c + 1) * BC])}
        m.update(consts)
        m.update(wis)
        in_maps.append(m)

    res = bass_utils.run_bass_kernel_spmd(nc, in_maps, core_ids=list(range(NCORES)))
    outs = [r["out"] for r in res.results]
    return np.concatenate(outs, axis=0).reshape(B, NCAP, DCAP).astype(np.float32)


# revision 16
# speedup vs baseline: 1.5669x; 1.5669x over previous
"""Trainium2 Bass kernel for CapsNet dynamic routing (nn_Capsule_13692355740297).

Math (per batch element):
    u_hat[i, (n,d)] = u[i, :] @ W[:, (n,d)]            # never materialized
    iter1: c uniform 1/10  -> s1 = 0.1 * (sum_i u_i)^T W
    iter k: b[i, n] = v_n . u_i   with v_n = W_n o_n   # contract Din on PE
            c = softmax_n(b)                           # free-dim softmax, [i,n] layout
            R[n, :] = sum_i c[i, n] u_i                # contract i on PE
            s[n, :] = R[n, :] @ W_n                    # small fixup matmuls
            o = squash(s)
Sharding: data-parallel over batch, 8 batch elements per core, no collectives.

SBUF per core: U f32 [128, 8*4096] (natural, i%128 on partitions) = 128KB/part
               UT bf16 [128, 8*4096] (Din on partitions)           = 64KB/part
UT is produced on-chip with PE identity-transposes (f32 DMA transpose is not
supported by the xbar), cast to bf16 in the PSUM->SBUF copy.
"""

import numpy as np

B, I_FULL, DIN = 64, 4096, 128
NCAP, DCAP = 10, 16
KND = NCAP * DCAP  # 160
NCORES = 8
BC = B // NCORES  # 8 batch elements per core
NT_FULL = I_FULL // 128  # 32 i-tiles per batch
EPS = 1e-7


def build_nc(bc=BC, nt=NT_FULL):
    import concourse.bacc as bacc
    import concourse.mybir as mybir
    from concourse.tile import TileContext

    fp32 = mybir.dt.float32
    bf16 = mybir.dt.bfloat16
    AX = mybir.AxisListType
    ALU = mybir.AluOpType
    ACTF = mybir.ActivationFunctionType

    il = nt * 128  # I per batch

    nc = bacc.Bacc(trn_type="TRN2")
    u_h = nc.dram_tensor("u", [bc, il, DIN], fp32, kind="ExternalInput")
    w_h = nc.dram_tensor("w", [DIN, KND], fp32, kind="ExternalInput")
    ident_h = nc.dram_tensor("ident", [128, 128], fp32, kind="ExternalInput")
    identb_h = nc.dram_tensor("identb", [128, 128], bf16, kind="ExternalInput")
    ones_h = nc.dram_tensor("ones", [128, 1], fp32, kind="ExternalInput")
    wt_hi_h = nc.dram_tensor("wt_hi", [128, DIN], bf16, kind="ExternalInput")
    wt_lo_h = nc.dram_tensor("wt_lo", [32, DIN], bf16, kind="ExternalInput")
    m_hi_h = nc.dram_tensor("m_hi", [128, NCAP], bf16, kind="ExternalInput")
    m_lo_h = nc.dram_tensor("m_lo", [32, NCAP], bf16, kind="ExternalInput")
    esel_h = nc.dram_tensor("esel", [128, bc * bc], fp32, kind="ExternalInput")
    out_h = nc.dram_tensor("out", [bc, KND], fp32, kind="ExternalOutput")

    with TileContext(nc) as tc:
        with (
            tc.tile_pool(name="big", bufs=1) as big,
            tc.tile_pool(name="sb2", bufs=2) as sb2,
            tc.tile_pool(name="psT", bufs=2, space="PSUM") as psT,
            tc.tile_pool(name="psB", bufs=2, space="PSUM") as psB,
            tc.tile_pool(name="psR", bufs=2, space="PSUM") as psR,
            tc.tile_pool(name="psS", bufs=2, space="PSUM") as psS,
        ):
            # ---------- persistent SBUF ----------
            U = big.tile([128, bc * il], bf16, name="U_sb")       # [i%128, (b,j,d)]
            UT = big.tile([128, bc * il], bf16, name="UT_sb")     # [d, (b,i)]
            W_sb = big.tile([128, KND], fp32, name="W_sb")
            ident = big.tile([128, 128], fp32, name="ident_sb")
            identb = big.tile([128, 128], bf16, name="identb_sb")
            ones = big.tile([128, 1], fp32, name="ones_sb")
            wt_hi = big.tile([128, DIN], bf16, name="wt_hi_sb")
            wt_lo = big.tile([32, DIN], bf16, name="wt_lo_sb")
            m_hi = big.tile([128, NCAP], bf16, name="m_hi_sb")
            m_lo = big.tile([32, NCAP], bf16, name="m_lo_sb")
            esel = big.tile([128, bc * bc], fp32, name="esel_sb")
            r0 = big.tile([128, bc], fp32, name="r0_sb")
            s_all = big.tile([bc, KND], fp32, name="s_all_sb")
            o_all = big.tile([bc, KND], fp32, name="o_all_sb")

            # ---------- constant loads ----------
            nc.sync.dma_start(out=W_sb[:, :], in_=w_h.ap())
            nc.sync.dma_start(out=ident[:, :], in_=ident_h.ap())
            nc.sync.dma_start(out=identb[:, :], in_=identb_h.ap())
            nc.sync.dma_start(out=ones[:, :], in_=ones_h.ap())
            nc.sync.dma_start(out=wt_hi[:, :], in_=wt_hi_h.ap())
            nc.sync.dma_start(out=wt_lo[:, :], in_=wt_lo_h.ap())
            nc.sync.dma_start(out=m_hi[:, :], in_=m_hi_h.ap())
            nc.sync.dma_start(out=m_lo[:, :], in_=m_lo_h.ap())
            nc.sync.dma_start(out=esel[:, :], in_=esel_h.ap())

            Uv = U[:, :].rearrange("p (b j d) -> p b j d", b=bc, j=nt, d=128)
            UTv = UT[:, :].rearrange("p (b i) -> p b i", b=bc, i=il)

            # ---------- bulk load: u[b] as [128, nt, 128] (i%128 on partitions) ----------
            h = nt // 2
            for b in range(bc):
                uin = u_h.ap()[b].rearrange("(j p) d -> p j d", p=128)
                nc.gpsimd.dma_start(out=Uv[:, b, :h], in_=uin[:, :h])
                nc.gpsimd.dma_start(out=Uv[:, b, h:], in_=uin[:, h:])

            # ---------- phase 1: build UT (PE transpose + cast), r0, s1 ----------
            for b in range(bc):
                for j in range(nt):
                    tp = psT.tile([128, 128], bf16, name="tp", tag="tp")
                    nc.tensor.transpose(tp[:, :], Uv[:, b, j], identb[:, :])
                    dst = UTv[:, b, 128 * j : 128 * (j + 1)]
                    if j % 2 == 0:
                        nc.scalar.copy(out=dst, in_=tp[:, :])
                    else:
                        nc.vector.tensor_copy(out=dst, in_=tp[:, :])
                # r0[:, b] = sum_i u_i (from UT, free-dim reduce)
                nc.vector.reduce_sum(
                    out=r0[:, b : b + 1], in_=UTv[:, b], axis=AX.X, op=ALU.add
                )
            # s1 (all batches at once) = 0.1 * r0^T W -> [bc, 160]
            s1p = psS.tile([bc, KND], fp32, name="s1p", tag="sacc")
            nc.tensor.matmul(s1p[:, :], r0[:, :], W_sb[:, :])
            nc.scalar.mul(out=s_all[:, :], in_=s1p[:, :], mul=0.1)

            def squash(it):
                """s_all -> o_all, in place over [bc, 160]."""
                sq = sb2.tile([bc, KND], fp32, name=f"sq{it}", tag="sq")
                q = sb2.tile([bc, NCAP], fp32, name=f"q{it}", tag="q")
                rt = sb2.tile([bc, NCAP], fp32, name=f"rt{it}", tag="rt")
                den = sb2.tile([bc, NCAP], fp32, name=f"den{it}", tag="den")
                rden = sb2.tile([bc, NCAP], fp32, name=f"rden{it}", tag="rden")
                coef = sb2.tile([bc, NCAP], fp32, name=f"coef{it}", tag="coef")
                nc.vector.tensor_tensor(
                    out=sq[:, :], in0=s_all[:, :], in1=s_all[:, :], op=ALU.mult
                )
                nc.vector.reduce_sum(
                    out=q[:, :],
                    in_=sq[:, :].rearrange("b (n d) -> b n d", n=NCAP),
                    axis=AX.X,
                    op=ALU.add,
                )
                nc.vector.tensor_scalar_add(q[:, :], q[:, :], EPS)
                nc.scalar.sqrt(out=rt[:, :], in_=q[:, :])
                nc.vector.tensor_scalar_add(den[:, :], q[:, :], 1.0)
                nc.vector.reciprocal(out=rden[:, :], in_=den[:, :])
                nc.vector.tensor_tensor(
                    out=coef[:, :], in0=rt[:, :], in1=rden[:, :], op=ALU.mult
                )
                nc.vector.tensor_tensor(
                    out=o_all[:, :].rearrange("b (n d) -> b n d", n=NCAP),
                    in0=s_all[:, :].rearrange("b (n d) -> b n d", n=NCAP),
                    in1=coef[:, :].unsqueeze(2).broadcast_to([bc, NCAP, DCAP]),
                    op=ALU.mult,
                )

            def make_V(it):
                """o_all -> V_sb bf16 [128, bc*10] (V^T per batch)."""
                oth_p = psT.tile([128, bc], fp32, name=f"oth{it}", tag="tp")
                otl_p = psT.tile([32, bc], fp32, name=f"otl{it}", tag="tp")
                nc.tensor.transpose(oth_p[:, :], o_all[:, 0:128], ident[:bc, :bc])
                nc.tensor.transpose(otl_p[:, :], o_all[:, 128:KND], ident[:bc, :bc])
                oth = sb2.tile([128, bc], bf16, name=f"oth_s{it}", tag="oth_s")
                otl = sb2.tile([32, bc], bf16, name=f"otl_s{it}", tag="otl_s")
                nc.scalar.copy(out=oth[:, :], in_=oth_p[:, :])
                nc.scalar.copy(out=otl[:, :], in_=otl_p[:, :])
                # Oexp[k, (b,n)] = oT[k, b] * M[k, n]
                oeh = sb2.tile([128, bc * NCAP], bf16, name=f"oeh{it}", tag="oeh")
                oel = sb2.tile([32, bc * NCAP], bf16, name=f"oel{it}", tag="oel")
                nc.vector.tensor_tensor(
                    out=oeh[:, :].rearrange("p (b n) -> p b n", b=bc),
                    in0=oth[:, :].unsqueeze(2).broadcast_to([128, bc, NCAP]),
                    in1=m_hi[:, :].unsqueeze(1).broadcast_to([128, bc, NCAP]),
                    op=ALU.mult,
                )
                nc.vector.tensor_tensor(
                    out=oel[:, :].rearrange("p (b n) -> p b n", b=bc),
                    in0=otl[:, :].unsqueeze(2).broadcast_to([32, bc, NCAP]),
                    in1=m_lo[:, :].unsqueeze(1).broadcast_to([32, bc, NCAP]),
                    op=ALU.mult,
                )
                vp = psT.tile([128, bc * NCAP], fp32, name=f"vp{it}", tag="tp")
                nc.tensor.matmul(vp[:, :], wt_hi[:, :], oeh[:, :], start=True, stop=False)
                nc.tensor.matmul(vp[:, :], wt_lo[:, :], oel[:, :], start=False, stop=True)
                V = sb2.tile([128, bc * NCAP], bf16, name=f"V{it}", tag="V")
                nc.scalar.copy(out=V[:, :], in_=vp[:, :])
                return V

            squash(1)

            # ---------- routing iterations 2..3 ----------
            for it in (2, 3):
                V = make_V(it)
                sp_all = psS.tile([bc, KND], fp32, name=f"sp{it}", tag="sacc")
                for b in range(bc):
                    # agreement logits b^T[i, n], tiled over i
                    btp = psB.tile([128, nt * NCAP], fp32, name=f"btp{it}", tag="btp")
                    for j in range(nt):
                        nc.tensor.matmul(
                            btp[:, NCAP * j : NCAP * (j + 1)],
                            UTv[:, b, 128 * j : 128 * (j + 1)],
                            V[:, NCAP * b : NCAP * (b + 1)],
                        )
                    # softmax over n (free dim), no max-subtraction (|b| bounded)
                    eb = sb2.tile([128, nt * NCAP], fp32, name=f"eb{it}", tag="eb")
                    nc.scalar.activation(eb[:, :], btp[:, :], ACTF.Exp)
                    ebv = eb[:, :].rearrange("p (j n) -> p j n", j=nt)
                    Z = sb2.tile([128, nt], fp32, name=f"Z{it}", tag="Z")
                    nc.vector.reduce_sum(out=Z[:, :], in_=ebv, axis=AX.X, op=ALU.add)
                    rZ = sb2.tile([128, nt], fp32, name=f"rZ{it}", tag="rZ")
                    nc.vector.reciprocal(out=rZ[:, :], in_=Z[:, :])
                    cc = sb2.tile([128, nt * NCAP], bf16, name=f"cc{it}", tag="cc")
                    nc.vector.tensor_tensor(
                        out=cc[:, :].rearrange("p (j n) -> p j n", j=nt),
                        in0=ebv,
                        in1=rZ[:, :].unsqueeze(2).broadcast_to([128, nt, NCAP]),
                        op=ALU.mult,
                    )
                    # R[n, :] = sum_i c[i, n] u_i  (PSUM-accumulated over i tiles)
                    Rp = psR.tile([NCAP, DIN], fp32, name=f"Rp{it}", tag="Rp")
                    for j in range(nt):
                        nc.tensor.matmul(
                            Rp[:, :],
                            cc[:, NCAP * j : NCAP * (j + 1)],
                            Uv[:, b, j],
                            start=(j == 0),
                            stop=(j == nt - 1),
                        )
                    R_sb = sb2.tile([NCAP, DIN], fp32, name=f"R{it}", tag="R")
                    nc.scalar.copy(out=R_sb[:, :], in_=Rp[:, :])
                    RTp = psT.tile([128, NCAP], fp32, name=f"RTp{it}", tag="tp")
                    nc.tensor.transpose(RTp[:, :], R_sb[:, :], ident[:NCAP, :NCAP])
                    RT = sb2.tile([128, NCAP], fp32, name=f"RT{it}", tag="RT")
                    nc.scalar.copy(out=RT[:, :], in_=RTp[:, :])
                    # s[(n,d)] = sum_d' RT[d', n] W[d', (n,d)]
                    prod = sb2.tile([128, KND], fp32, name=f"prod{it}", tag="prod")
                    nc.vector.tensor_tensor(
                        out=prod[:, :].rearrange("p (n d) -> p n d", n=NCAP),
                        in0=RT[:, :].unsqueeze(2).broadcast_to([128, NCAP, DCAP]),
                        in1=W_sb[:, :].rearrange("p (n d) -> p n d", n=NCAP),
                        op=ALU.mult,
                    )
                    nc.tensor.matmul(
                        sp_all[:, :],
                        esel[:, bc * b : bc * (b + 1)],
                        prod[:, :],
                        start=(b == 0),
                        stop=(b == bc - 1),
                    )
                nc.scalar.copy(out=s_all[:, :], in_=sp_all[:, :])
                squash(it)

            # ---------- store ----------
            nc.sync.dma_start(out=out_h.ap(), in_=o_all[:, :])

    nc.compile()
    return nc


def make_const_inputs(bc=BC):
    import ml_dtypes

    ident = np.eye(128, dtype=np.float32)
    ones = np.ones((128, 1), dtype=np.float32)
    mask = np.zeros((KND, NCAP), dtype=np.float32)
    for k in range(KND):
        mask[k, k // DCAP] = 1.0
    esel = np.zeros((128, bc * bc), dtype=np.float32)
    for b in range(bc):
        esel[:, b * bc + b] = 1.0
    return {
        "ident": ident,
        "identb": ident.astype(ml_dtypes.bfloat16),
        "ones": ones,
        "m_hi": mask[:128].astype(ml_dtypes.bfloat16),
        "m_lo": mask[128:].astype(ml_dtypes.bfloat16),
        "esel": esel,
    }


def make_w_inputs(W):
    import ml_dtypes

    W = np.asarray(W, dtype=np.float32)
    WT = W.T.copy()  # [160, 128]
    return {
        "w": W,
        "wt_hi": WT[:128].astype(ml_dtypes.bfloat16),
        "wt_lo": WT[128:].astype(ml_dtypes.bfloat16),
    }


_CACHE = {}


def kernel(u_vecs, W):
    from concourse import bass_utils

    u_vecs = np.asarray(u_vecs, dtype=np.float32)
    W = np.asarray(W, dtype=np.float32)
    if "nc" not in _CACHE:
        _CACHE["nc"] = build_nc()
    nc = _CACHE["nc"]

    consts = make_const_inputs()
    wis = make_w_inputs(W)
    in_maps = []
    for c in range(NCORES):
        m = {"u": np.ascontiguousarray(u_vecs[c * BC : (c + 1) * BC])}
        m.update(consts)
        m.update(wis)
        in_maps.append(m)

    res = bass_utils.run_bass_kernel_spmd(nc, in_maps, core_ids=list(range(NCORES)))
    outs = [r["out"] for r in res.results]
    return np.concatenate(outs, axis=0).reshape(B, NCAP, DCAP).astype(np.float32)


# revision 18
# speedup vs baseline: 2.3941x; 1.5279x over previous
"""Trainium2 Bass kernel for CapsNet dynamic routing (nn_Capsule_13692355740297).

Math (per batch element):
    u_hat[i, (n,d)] = u[i, :] @ W[:, (n,d)]            # never materialized
    iter1: c uniform 1/10  -> s1 = 0.1 * (sum_i u_i)^T W
    iter k: b[i, n] = v_n . u_i   with v_n = W_n o_n   # contract Din on PE
            c = softmax_n(b)                           # free-dim softmax, [i,n] layout
            R[n, :] = sum_i c[i, n] u_i                # contract i on PE
            s[n, :] = R[n, :] @ W_n                    # small fixup matmuls
            o = squash(s)
Sharding: data-parallel over batch, 8 batch elements per core, no collectives.

SBUF per core: U f32 [128, 8*4096] (natural, i%128 on partitions) = 128KB/part
               UT bf16 [128, 8*4096] (Din on partitions)           = 64KB/part
UT is produced on-chip with PE identity-transposes (f32 DMA transpose is not
supported by the xbar), cast to bf16 in the PSUM->SBUF copy.
"""

import numpy as np

B, I_FULL, DIN = 64, 4096, 128
NCAP, DCAP = 10, 16
KND = NCAP * DCAP  # 160
NCORES = 8
BC = B // NCORES  # 8 batch elements per core
NT_FULL = I_FULL // 128  # 32 i-tiles per batch
EPS = 1e-7


def build_nc(bc=BC, nt=NT_FULL):
    import concourse.bacc as bacc
    import concourse.mybir as mybir
    from concourse.tile import TileContext

    fp32 = mybir.dt.float32
    bf16 = mybir.dt.bfloat16
    AX = mybir.AxisListType
    ALU = mybir.AluOpType
    ACTF = mybir.ActivationFunctionType

    il = nt * 128  # I per batch

    nc = bacc.Bacc(trn_type="TRN2")
    u_h = nc.dram_tensor("u", [bc, il, DIN], fp32, kind="ExternalInput")
    w_h = nc.dram_tensor("w", [DIN, KND], fp32, kind="ExternalInput")
    ident_h = nc.dram_tensor("ident", [128, 128], fp32, kind="ExternalInput")
    identb_h = nc.dram_tensor("identb", [128, 128], bf16, kind="ExternalInput")
    ones_h = nc.dram_tensor("ones", [128, 1], fp32, kind="ExternalInput")
    wt_hi_h = nc.dram_tensor("wt_hi", [128, DIN], bf16, kind="ExternalInput")
    wt_lo_h = nc.dram_tensor("wt_lo", [32, DIN], bf16, kind="ExternalInput")
    m_hi_h = nc.dram_tensor("m_hi", [128, NCAP], bf16, kind="ExternalInput")
    m_lo_h = nc.dram_tensor("m_lo", [32, NCAP], bf16, kind="ExternalInput")
    esel_h = nc.dram_tensor("esel", [128, bc * bc], fp32, kind="ExternalInput")
    out_h = nc.dram_tensor("out", [bc, KND], fp32, kind="ExternalOutput")

    with TileContext(nc) as tc:
        with (
            tc.tile_pool(name="big", bufs=1) as big,
            tc.tile_pool(name="sb2", bufs=2) as sb2,
            tc.tile_pool(name="psT", bufs=2, space="PSUM") as psT,
            tc.tile_pool(name="psB", bufs=2, space="PSUM") as psB,
            tc.tile_pool(name="psR", bufs=2, space="PSUM") as psR,
            tc.tile_pool(name="psS", bufs=2, space="PSUM") as psS,
        ):
            # ---------- persistent SBUF ----------
            U = big.tile([128, bc * il], bf16, name="U_sb")       # [i%128, (b,j,d)]
            UT = big.tile([128, bc * il], bf16, name="UT_sb")     # [d, (b,i)]
            W_sb = big.tile([128, KND], fp32, name="W_sb")
            ident = big.tile([128, 128], fp32, name="ident_sb")
            identb = big.tile([128, 128], bf16, name="identb_sb")
            ones = big.tile([128, 1], fp32, name="ones_sb")
            wt_hi = big.tile([128, DIN], bf16, name="wt_hi_sb")
            wt_lo = big.tile([32, DIN], bf16, name="wt_lo_sb")
            m_hi = big.tile([128, NCAP], bf16, name="m_hi_sb")
            m_lo = big.tile([32, NCAP], bf16, name="m_lo_sb")
            esel = big.tile([128, bc * bc], fp32, name="esel_sb")
            r0 = big.tile([128, bc], fp32, name="r0_sb")
            s_all = big.tile([bc, KND], fp32, name="s_all_sb")
            o_all = big.tile([bc, KND], fp32, name="o_all_sb")

            # ---------- constant loads ----------
            nc.sync.dma_start(out=W_sb[:, :], in_=w_h.ap())
            nc.sync.dma_start(out=ident[:, :], in_=ident_h.ap())
            nc.sync.dma_start(out=identb[:, :], in_=identb_h.ap())
            nc.sync.dma_start(out=ones[:, :], in_=ones_h.ap())
            nc.sync.dma_start(out=wt_hi[:, :], in_=wt_hi_h.ap())
            nc.sync.dma_start(out=wt_lo[:, :], in_=wt_lo_h.ap())
            nc.sync.dma_start(out=m_hi[:, :], in_=m_hi_h.ap())
            nc.sync.dma_start(out=m_lo[:, :], in_=m_lo_h.ap())
            nc.sync.dma_start(out=esel[:, :], in_=esel_h.ap())

            Uv = U[:, :].rearrange("p (b j d) -> p b j d", b=bc, j=nt, d=128)
            UTv = UT[:, :].rearrange("p (b i) -> p b i", b=bc, i=il)

            # ---------- bulk load: u[b] as [128, nt, 128] (i%128 on partitions) ----------
            for b in range(bc):
                uin = u_h.ap()[b].rearrange("(j p) d -> p j d", p=128)
                nc.gpsimd.dma_start(out=Uv[:, b], in_=uin)

            # ---------- phase 1: build UT (PE transpose, 4-tile groups), r0 via accum ----------
            ng = nt // 4
            for b in range(bc):
                racc = sb2.tile([128, ng], fp32, name=f"racc{b}", tag="racc")
                for g in range(ng):
                    tp = psT.tile([128, 512], bf16, name="tp", tag="tp")
                    for jj in range(4):
                        nc.tensor.transpose(
                            tp[:, 128 * jj : 128 * (jj + 1)],
                            Uv[:, b, 4 * g + jj],
                            identb[:, :],
                        )
                    dst = UTv[:, b, 512 * g : 512 * (g + 1)]
                    if g % 2 == 0:
                        nc.scalar.activation(
                            dst, tp[:, :], ACTF.Copy, accum_out=racc[:, g : g + 1]
                        )
                    else:
                        nc.vector.tensor_scalar(
                            out=dst,
                            in0=tp[:, :],
                            scalar1=0.0,
                            scalar2=0.0,
                            op0=ALU.add,
                            op1=ALU.add,
                            accum_out=racc[:, g : g + 1],
                        )
                # r0[:, b] = sum of group partial sums
                nc.vector.reduce_sum(
                    out=r0[:, b : b + 1], in_=racc[:, :], axis=AX.X, op=ALU.add
                )
            # s1 (all batches at once) = 0.1 * r0^T W -> [bc, 160]
            s1p = psS.tile([bc, KND], fp32, name="s1p", tag="sacc")
            nc.tensor.matmul(s1p[:, :], r0[:, :], W_sb[:, :])
            nc.scalar.mul(out=s_all[:, :], in_=s1p[:, :], mul=0.1)

            def squash(it):
                """s_all -> o_all, in place over [bc, 160]."""
                sq = sb2.tile([bc, KND], fp32, name=f"sq{it}", tag="sq")
                q = sb2.tile([bc, NCAP], fp32, name=f"q{it}", tag="q")
                rt = sb2.tile([bc, NCAP], fp32, name=f"rt{it}", tag="rt")
                den = sb2.tile([bc, NCAP], fp32, name=f"den{it}", tag="den")
                rden = sb2.tile([bc, NCAP], fp32, name=f"rden{it}", tag="rden")
                coef = sb2.tile([bc, NCAP], fp32, name=f"coef{it}", tag="coef")
                nc.vector.tensor_tensor(
                    out=sq[:, :], in0=s_all[:, :], in1=s_all[:, :], op=ALU.mult
                )
                nc.vector.reduce_sum(
                    out=q[:, :],
                    in_=sq[:, :].rearrange("b (n d) -> b n d", n=NCAP),
                    axis=AX.X,
                    op=ALU.add,
                )
                nc.vector.tensor_scalar_add(q[:, :], q[:, :], EPS)
                nc.scalar.sqrt(out=rt[:, :], in_=q[:, :])
                nc.vector.tensor_scalar_add(den[:, :], q[:, :], 1.0)
                nc.vector.reciprocal(out=rden[:, :], in_=den[:, :])
                nc.vector.tensor_tensor(
                    out=coef[:, :], in0=rt[:, :], in1=rden[:, :], op=ALU.mult
                )
                nc.vector.tensor_tensor(
                    out=o_all[:, :].rearrange("b (n d) -> b n d", n=NCAP),
                    in0=s_all[:, :].rearrange("b (n d) -> b n d", n=NCAP),
                    in1=coef[:, :].unsqueeze(2).broadcast_to([bc, NCAP, DCAP]),
                    op=ALU.mult,
                )

            def make_V(it):
                """o_all -> V_sb bf16 [128, bc*10] (V^T per batch)."""
                oth_p = psT.tile([128, bc], fp32, name=f"oth{it}", tag="tp")
                otl_p = psT.tile([32, bc], fp32, name=f"otl{it}", tag="tp")
                nc.tensor.transpose(oth_p[:, :], o_all[:, 0:128], ident[:bc, :bc])
                nc.tensor.transpose(otl_p[:, :], o_all[:, 128:KND], ident[:bc, :bc])
                oth = sb2.tile([128, bc], bf16, name=f"oth_s{it}", tag="oth_s")
                otl = sb2.tile([32, bc], bf16, name=f"otl_s{it}", tag="otl_s")
                nc.scalar.copy(out=oth[:, :], in_=oth_p[:, :])
                nc.scalar.copy(out=otl[:, :], in_=otl_p[:, :])
                # Oexp[k, (b,n)] = oT[k, b] * M[k, n]
                oeh = sb2.tile([128, bc * NCAP], bf16, name=f"oeh{it}", tag="oeh")
                oel = sb2.tile([32, bc * NCAP], bf16, name=f"oel{it}", tag="oel")
                nc.vector.tensor_tensor(
                    out=oeh[:, :].rearrange("p (b n) -> p b n", b=bc),
                    in0=oth[:, :].unsqueeze(2).broadcast_to([128, bc, NCAP]),
                    in1=m_hi[:, :].unsqueeze(1).broadcast_to([128, bc, NCAP]),
                    op=ALU.mult,
                )
                nc.vector.tensor_tensor(
                    out=oel[:, :].rearrange("p (b n) -> p b n", b=bc),
                    in0=otl[:, :].unsqueeze(2).broadcast_to([32, bc, NCAP]),
                    in1=m_lo[:, :].unsqueeze(1).broadcast_to([32, bc, NCAP]),
                    op=ALU.mult,
                )
                vp = psT.tile([128, bc * NCAP], fp32, name=f"vp{it}", tag="tp")
                nc.tensor.matmul(vp[:, :], wt_hi[:, :], oeh[:, :], start=True, stop=False)
                nc.tensor.matmul(vp[:, :], wt_lo[:, :], oel[:, :], start=False, stop=True)
                V = sb2.tile([128, bc * NCAP], bf16, name=f"V{it}", tag="V")
                nc.scalar.copy(out=V[:, :], in_=vp[:, :])
                return V

            squash(1)

            # ---------- routing iterations 2..3 ----------
            for it in (2, 3):
                V = make_V(it)
                sp_all = psS.tile([bc, KND], fp32, name=f"sp{it}", tag="sacc")
                for b in range(bc):
                    # agreement logits b^T[i, n], tiled over i
                    btp = psB.tile([128, nt * NCAP], fp32, name=f"btp{it}", tag="btp")
                    for j in range(nt):
                        nc.tensor.matmul(
                            btp[:, NCAP * j : NCAP * (j + 1)],
                            UTv[:, b, 128 * j : 128 * (j + 1)],
                            V[:, NCAP * b : NCAP * (b + 1)],
                        )
                    # softmax over n (free dim), no max-subtraction (|b| bounded)
                    eb = sb2.tile([128, nt * NCAP], fp32, name=f"eb{it}", tag="eb")
                    nc.scalar.activation(eb[:, :], btp[:, :], ACTF.Exp)
                    ebv = eb[:, :].rearrange("p (j n) -> p j n", j=nt)
                    Z = sb2.tile([128, nt], fp32, name=f"Z{it}", tag="Z")
                    nc.vector.reduce_sum(out=Z[:, :], in_=ebv, axis=AX.X, op=ALU.add)
                    rZ = sb2.tile([128, nt], fp32, name=f"rZ{it}", tag="rZ")
                    nc.vector.reciprocal(out=rZ[:, :], in_=Z[:, :])
                    cc = sb2.tile([128, nt * NCAP], bf16, name=f"cc{it}", tag="cc")
                    nc.vector.tensor_tensor(
                        out=cc[:, :].rearrange("p (j n) -> p j n", j=nt),
                        in0=ebv,
                        in1=rZ[:, :].unsqueeze(2).broadcast_to([128, nt, NCAP]),
                        op=ALU.mult,
                    )
                    # R^T[d, n] = sum_i U[i, d] c[i, n]  (PSUM-accumulated over i tiles)
                    Rp = psR.tile([128, NCAP], fp32, name=f"Rp{it}", tag="Rp")
                    for j in range(nt):
                        nc.tensor.matmul(
                            Rp[:, :],
                            Uv[:, b, j],
                            cc[:, NCAP * j : NCAP * (j + 1)],
                            start=(j == 0),
                            stop=(j == nt - 1),
                        )
                    # s[(n,d)] = sum_d' RT[d', n] W[d', (n,d)]
                    prod = sb2.tile([128, KND], fp32, name=f"prod{it}", tag="prod")
                    nc.vector.tensor_tensor(
                        out=prod[:, :].rearrange("p (n d) -> p n d", n=NCAP),
                        in0=Rp[:, :].unsqueeze(2).broadcast_to([128, NCAP, DCAP]),
                        in1=W_sb[:, :].rearrange("p (n d) -> p n d", n=NCAP),
                        op=ALU.mult,
                    )
                    nc.tensor.matmul(
                        sp_all[:, :],
                        esel[:, bc * b : bc * (b + 1)],
                        prod[:, :],
                        start=(b == 0),
                        stop=(b == bc - 1),
                    )
                nc.scalar.copy(out=s_all[:, :], in_=sp_all[:, :])
                squash(it)

            # ---------- store ----------
            nc.sync.dma_start(out=out_h.ap(), in_=o_all[:, :])

    nc.compile()
    return nc


def make_const_inputs(bc=BC):
    import ml_dtypes

    ident = np.eye(128, dtype=np.float32)
    ones = np.ones((128, 1), dtype=np.float32)
    mask = np.zeros((KND, NCAP), dtype=np.float32)
    for k in range(KND):
        mask[k, k // DCAP] = 1.0
    esel = np.zeros((128, bc * bc), dtype=np.float32)
    for b in range(bc):
        esel[:, b * bc + b] = 1.0
    return {
        "ident": ident,
        "identb": ident.astype(ml_dtypes.bfloat16),
        "ones": ones,
        "m_hi": mask[:128].astype(ml_dtypes.bfloat16),
        "m_lo": mask[128:].astype(ml_dtypes.bfloat16),
        "esel": esel,
    }


def make_w_inputs(W):
    import ml_dtypes

    W = np.asarray(W, dtype=np.float32)
    WT = W.T.copy()  # [160, 128]
    return {
        "w": W,
        "wt_hi": WT[:128].astype(ml_dtypes.bfloat16),
        "wt_lo": WT[128:].astype(ml_dtypes.bfloat16),
    }


_CACHE = {}


def kernel(u_vecs, W):
    from concourse import bass_utils

    u_vecs = np.asarray(u_vecs, dtype=np.float32)
    W = np.asarray(W, dtype=np.float32)
    if "nc" not in _CACHE:
        _CACHE["nc"] = build_nc()
    nc = _CACHE["nc"]

    consts = make_const_inputs()
    wis = make_w_inputs(W)
    in_maps = []
    for c in range(NCORES):
        m = {"u": np.ascontiguousarray(u_vecs[c * BC : (c + 1) * BC])}
        m.update(consts)
        m.update(wis)
        in_maps.append(m)

    res = bass_utils.run_bass_kernel_spmd(nc, in_maps, core_ids=list(range(NCORES)))
    outs = [r["out"] for r in res.results]
    return np.concatenate(outs, axis=0).reshape(B, NCAP, DCAP).astype(np.float32)
